# revision 23
# baseline (speedup 1.0000x reference)
"""AttentiveStatPooling Trainium2 kernel (8-core SPMD, data-parallel over batch).

Contract: kernel(**inputs) takes the FULL unsharded inputs (as produced by
reference.setup_inputs()) and returns the FULL [B, 2C] output.

Math (per sample, identical to the jax reference):
  mean/std over T of x;  h = relu(Wx@x + (Wm@mean + Ws@std + b1));
  g = tanh(BN1(h));  l = BN2scale * relu(W2@g + b2)  (the BN2 shift cancels in
  the softmax and is dropped);  w = softmax(l, axis=T);
  out = [sum(x*w), sqrt(clip(sum(x^2*w) - mu^2, 1e-4))].

Implementation notes:
  - batch 32 split 4 samples/core across 8 NeuronCores (pure DP).
  - x shipped in bf16 (halves DMA; matmuls/weighted sums read bf16, all
    reductions accumulate fp32 on-engine via accum_out).
  - BN affines folded into per-partition ACT scale/bias vectors (host-side).
  - softmax needs no max-subtraction (logits bounded, per-row shift cancels);
    relu inside the softmax realized as max(exp(l), 1).
  - sqrt via Newton/rsqrt on the vector engine (avoids ACT table switches).
  - emission is software-pipelined: phase A of sample s+2 and phase B of
    sample s+1 are interleaved into phase C of sample s so every engine's
    in-order instruction stream stays busy.
"""

import numpy as np
import ml_dtypes

B, C, T, A = 32, 1536, 1000, 128
N_CORES = 8
SPC = B // N_CORES        # samples per core
NCH = C // 128            # 12 channel chunks of 128
BN_EPS = 1e-5
CLAMP = 1e-4
HALVES = ((0, 512), (512, 1000))   # psum-bank-aligned split of T

_CACHE = {}


def _build_module(loop_reps=1):
    import concourse.tile as tile
    from concourse import bacc, mybir
    from contextlib import ExitStack

    f32, bf16 = mybir.dt.float32, mybir.dt.bfloat16
    Alu = mybir.AluOpType
    Act = mybir.ActivationFunctionType

    nc = bacc.Bacc("TRN2", target_bir_lowering=False, debug=False,
                   num_devices=N_CORES)

    xbf = nc.dram_tensor("xbf", [SPC, C, T], bf16, kind="ExternalInput").ap()
    w1xT = nc.dram_tensor("w1xT", [C, A], bf16, kind="ExternalInput").ap()
    wmsT = nc.dram_tensor("wmsT", [2 * C, A], f32, kind="ExternalInput").ap()
    w2T = nc.dram_tensor("w2T", [A, C], bf16, kind="ExternalInput").ap()
    b1d = nc.dram_tensor("b1d", [A, 1], f32, kind="ExternalInput").ap()
    inv1d = nc.dram_tensor("inv1d", [A, 1], f32, kind="ExternalInput").ap()
    add1d = nc.dram_tensor("add1d", [A, 1], f32, kind="ExternalInput").ap()
    inv2d = nc.dram_tensor("inv2d", [128, NCH], f32, kind="ExternalInput").ap()
    b2pd = nc.dram_tensor("b2pd", [128, NCH], f32, kind="ExternalInput").ap()
    identd = nc.dram_tensor("identd", [128, 128], f32, kind="ExternalInput").ap()
    out = nc.dram_tensor("out", [SPC, 2 * C], f32, kind="ExternalOutput").ap()

    with tile.TileContext(nc) as tc:
        with ExitStack() as ctx:
            cpool = ctx.enter_context(tc.tile_pool(name="const", bufs=1))
            xpool = ctx.enter_context(tc.tile_pool(name="x", bufs=14))
            epool = ctx.enter_context(tc.tile_pool(name="e", bufs=3))
            ebpool = ctx.enter_context(tc.tile_pool(name="eb", bufs=3))
            ppool = ctx.enter_context(tc.tile_pool(name="p", bufs=3))
            jpool = ctx.enter_context(tc.tile_pool(name="junk", bufs=6))
            rpool = ctx.enter_context(tc.tile_pool(name="r", bufs=2))
            gpool = ctx.enter_context(tc.tile_pool(name="g", bufs=2))
            spool = ctx.enter_context(tc.tile_pool(name="stats", bufs=3))
            smpool = ctx.enter_context(tc.tile_pool(name="small", bufs=8))
            opool = ctx.enter_context(tc.tile_pool(name="ostage", bufs=4))
            ph1p = ctx.enter_context(tc.tile_pool(name="ph1", bufs=1, space="PSUM"))
            p2p = ctx.enter_context(tc.tile_pool(name="p2", bufs=2, space="PSUM"))
            pmvp = ctx.enter_context(tc.tile_pool(name="pmv", bufs=1, space="PSUM"))
            ptrp = ctx.enter_context(tc.tile_pool(name="ptr", bufs=1, space="PSUM"))

            st = {}   # per-sample state

            def dma_x(s, groups=range(4)):
                if s not in st:
                    st[s] = {"xg": [], "x": []}
                for g in groups:
                    xt = xpool.tile([128, 3 * T], bf16, name="x", tag="x")
                    src_ap = xbf[s, g * 384:(g + 1) * 384, :]
                    src_ap = src_ap.rearrange("(c p) t -> p c t", p=128)
                    nc.sync.dma_start(xt[:].rearrange("p (c t) -> p c t", t=T), src_ap)
                    st[s]["xg"].append(xt)
                    for i in range(3):
                        st[s]["x"].append(xt[:, i * T:(i + 1) * T])

            def phaseA_moments(s, c, sxx_dve=False):
                """Sx/Sxx accumulation for chunk c of sample s."""
                d = st[s]
                if c == 0:
                    d["sx"] = spool.tile([128, NCH], f32, name="sx", tag="sx")
                    d["sxx"] = spool.tile([128, NCH], f32, name="sxx", tag="sxx")
                xt = d["x"][c]
                j1 = jpool.tile([128, T], bf16, name="junk", tag="junk")
                nc.vector.tensor_scalar(j1[:], xt, 0.0, 0.0, Alu.add,
                                        Alu.add, accum_out=d["sx"][:, c:c + 1])
                j2 = jpool.tile([128, T], bf16, name="junk", tag="junk")
                if sxx_dve:
                    nc.vector.tensor_tensor(j2[:], xt, xt, Alu.mult)
                    j3 = jpool.tile([128, T], bf16, name="junk", tag="junk")
                    nc.vector.tensor_scalar(j3[:], j2[:], 0.0, 0.0, Alu.add,
                                            Alu.add, accum_out=d["sxx"][:, c:c + 1])
                else:
                    nc.scalar.activation(j2[:], xt, Act.Square,
                                         accum_out=d["sxx"][:, c:c + 1])

            def phaseA_mm1(s, c):
                d = st[s]
                if c == 0:
                    d["ph1"] = ph1p.tile([A, T], f32, name="ph1", tag="ph1")
                xt = d["x"][c]
                for lo, hi in HALVES:
                    nc.tensor.matmul(d["ph1"][:, lo:hi], w1xT_t[c],
                                     xt[:, lo:hi], start=(c == 0),
                                     stop=(c == NCH - 1), skip_group_check=True)

            def newton_rsqrt(v_ap, out_ap, n, iters):
                """out = 1/sqrt(v) elementwise on a [128, n] fp32 AP."""
                t0 = smpool.tile([128, n], f32, name="nw0", tag="nw0")
                t1 = smpool.tile([128, n], f32, name="nw1", tag="nw1")
                r = smpool.tile([128, n], f32, name="nwr", tag="nwr")
                nc.vector.tensor_scalar(t0[:], v_ap, 0.5, 0.5, Alu.mult, Alu.add)
                nc.vector.reciprocal(r[:], t0[:])
                for it in range(iters):
                    dst = out_ap if it == iters - 1 else r[:]
                    nc.vector.tensor_tensor(t0[:], v_ap, r[:], Alu.mult)
                    nc.vector.tensor_tensor(t1[:], t0[:], r[:], Alu.mult)
                    nc.vector.tensor_scalar(t0[:], t1[:], -0.5, 1.5, Alu.mult, Alu.add)
                    nc.vector.tensor_tensor(dst, r[:], t0[:], Alu.mult)

            def phaseB_stats(s):
                """mean/std + mean-half of the bias matvec."""
                d = st[s]
                ms = smpool.tile([128, 2 * NCH], f32, name="ms", tag="ms")
                d["ms"] = ms
                mean_ap = ms[:, 0:NCH]
                std_ap = ms[:, NCH:2 * NCH]
                nc.vector.tensor_scalar(mean_ap, d["sx"][:], 1.0 / T, None, Alu.mult)
                pmv = pmvp.tile([A, 1], f32, name="pmv", tag="pmv")
                d["pmv"] = pmv
                for k in range(NCH):
                    nc.tensor.matmul(pmv[:], wms_t[k], ms[:, k:k + 1],
                                     start=(k == 0), stop=False,
                                     skip_group_check=True)
                m2 = smpool.tile([128, NCH], f32, name="m2", tag="m2")
                nc.vector.tensor_tensor(m2[:], d["sx"][:], d["sx"][:], Alu.mult)
                var = smpool.tile([128, NCH], f32, name="var", tag="var")
                nc.vector.scalar_tensor_tensor(var[:], m2[:], -1.0 / T, d["sxx"][:],
                                               Alu.mult, Alu.add)
                v = smpool.tile([128, NCH], f32, name="v", tag="v")
                nc.vector.tensor_scalar(v[:], var[:], 1.0 / (T - 1), CLAMP,
                                        Alu.mult, Alu.max)
                rs = smpool.tile([128, NCH], f32, name="rs", tag="rs")
                newton_rsqrt(v[:], rs[:], NCH, 3)
                nc.vector.tensor_tensor(std_ap, v[:], rs[:], Alu.mult)

            def phaseB_main(s):
                """std-half of the matvec -> relu -> tanh (g)."""
                d = st[s]
                ms, pmv = d["ms"], d["pmv"]
                for k in range(NCH, 2 * NCH):
                    nc.tensor.matmul(pmv[:], wms_t[k], ms[:, k:k + 1],
                                     start=False, stop=(k == 2 * NCH - 1),
                                     skip_group_check=True)
                btot = smpool.tile([A, 1], f32, name="btot", tag="btot")
                nc.vector.tensor_tensor(btot[:], pmv[:], b1_t[:], Alu.add)
                rt = rpool.tile([A, T], f32, name="r", tag="r")
                nc.scalar.activation(rt[:], d["ph1"][:], Act.Relu, bias=btot[:])
                gt = gpool.tile([A, T], bf16, name="g", tag="g")
                nc.scalar.activation(gt[:], rt[:], Act.Tanh, bias=add1_t[:],
                                     scale=inv1_t[:])
                d["g"] = gt

            def phaseB(s):
                phaseB_stats(s)
                phaseB_main(s)

            def phaseC_chunk(s, c, s2_act=False):
                d = st[s]
                if c == 0:
                    d["S0"] = spool.tile([128, NCH], f32, name="S0", tag="S0")
                    d["S1"] = spool.tile([128, NCH], f32, name="S1", tag="S1")
                    d["S2"] = spool.tile([128, NCH], f32, name="S2", tag="S2")
                p2 = p2p.tile([128, T], f32, name="p2", tag="p2")
                wsl = w2T_t[:, c * 128:(c + 1) * 128]
                for lo, hi in HALVES:
                    nc.tensor.matmul(p2[:, lo:hi], wsl, d["g"][:, lo:hi],
                                     start=True, stop=True)
                E = epool.tile([128, T], bf16, name="E", tag="E")
                nc.scalar.activation(E[:], p2[:], Act.Exp,
                                     bias=b2p_t[:, c:c + 1], scale=inv2_t[:, c:c + 1])
                eb = ebpool.tile([128, T], bf16, name="eb", tag="eb")
                nc.vector.tensor_scalar(eb[:], E[:], 1.0, 0.0, Alu.max,
                                        Alu.add, accum_out=d["S0"][:, c:c + 1])
                xt = d["x"][c]
                pt = ppool.tile([128, T], bf16, name="p", tag="p")
                nc.vector.tensor_tensor(pt[:], eb[:], xt, Alu.mult)
                j1 = jpool.tile([128, T], bf16, name="junk", tag="junk")
                nc.vector.tensor_scalar(j1[:], pt[:], 0.0, 0.0, Alu.add,
                                        Alu.add, accum_out=d["S1"][:, c:c + 1])
                qt = ppool.tile([128, T], bf16, name="p", tag="p")
                nc.vector.tensor_tensor(qt[:], pt[:], xt, Alu.mult)
                j2 = jpool.tile([128, T], bf16, name="junk", tag="junk")
                if s2_act:
                    nc.scalar.activation(j2[:], qt[:], Act.Identity,
                                         accum_out=d["S2"][:, c:c + 1])
                else:
                    nc.vector.tensor_scalar(j2[:], qt[:], 0.0, 0.0, Alu.add,
                                            Alu.add, accum_out=d["S2"][:, c:c + 1])

            def sample_out(s):
                """mu/sg + transpose (DVE 32x32 blocks) + store."""
                d = st[s]
                rc = smpool.tile([128, NCH], f32, name="rc", tag="rc")
                nc.vector.reciprocal(rc[:], d["S0"][:])
                mu = opool.tile([128, NCH], f32, name="mu", tag="mu")
                sg = opool.tile([128, NCH], f32, name="sg", tag="sg")
                nc.vector.tensor_tensor(mu[:], d["S1"][:], rc[:], Alu.mult)
                ex2 = smpool.tile([128, NCH], f32, name="ex2", tag="ex2")
                nc.vector.tensor_tensor(ex2[:], d["S2"][:], rc[:], Alu.mult)
                mu2 = smpool.tile([128, NCH], f32, name="mu2", tag="mu2")
                nc.vector.tensor_tensor(mu2[:], mu[:], mu[:], Alu.mult)
                sg2 = smpool.tile([128, NCH], f32, name="sg2", tag="sg2")
                nc.vector.scalar_tensor_tensor(sg2[:], mu2[:], -1.0, ex2[:],
                                               Alu.mult, Alu.add)
                v2 = smpool.tile([128, NCH], f32, name="v2", tag="v2")
                nc.vector.tensor_scalar(v2[:], sg2[:], 1.0, CLAMP, Alu.mult, Alu.max)
                rsg = smpool.tile([128, NCH], f32, name="rsg", tag="rsg")
                newton_rsqrt(v2[:], rsg[:], NCH, 6)
                nc.vector.tensor_tensor(sg[:], v2[:], rsg[:], Alu.mult)
                for half, srct in ((0, mu), (1, sg)):
                    ptr = ptrp.tile([NCH, 128], f32, name="ptr", tag="ptr")
                    nc.tensor.transpose(ptr[:], srct[:], ident_t[:])
                    ost = opool.tile([NCH, 128], f32, name="ost", tag="ost")
                    nc.scalar.copy(ost[:], ptr[:])
                    dst = out[s, half * C:(half + 1) * C]
                    dst = dst.rearrange("(ci p) -> ci p", p=128)
                    nc.sync.dma_start(dst, ost[:])

            # ---------------- constant loads (interleaved with x below) ----
            def load_w1xT():
                t = cpool.tile([128, NCH * A], bf16, name="w1xall", tag="w1xall")
                src_ap = w1xT.rearrange("(c p) a -> p c a", p=128)
                nc.sync.dma_start(t[:].rearrange("p (c a) -> p c a", a=A), src_ap)
                return [t[:, c * A:(c + 1) * A] for c in range(NCH)]

            def load_params():
                global b1_t, inv1_t, add1_t, inv2_t, b2p_t, w2T_t, wms_t, ident_t
                b1_t = cpool.tile([A, 1], f32, name="b1", tag="b1")
                nc.sync.dma_start(b1_t[:], b1d[:])
                inv1_t = cpool.tile([A, 1], f32, name="inv1", tag="inv1")
                nc.sync.dma_start(inv1_t[:], inv1d[:])
                add1_t = cpool.tile([A, 1], f32, name="add1", tag="add1")
                nc.sync.dma_start(add1_t[:], add1d[:])
                inv2_t = cpool.tile([128, NCH], f32, name="inv2", tag="inv2")
                nc.sync.dma_start(inv2_t[:], inv2d[:])
                b2p_t = cpool.tile([128, NCH], f32, name="b2p", tag="b2p")
                nc.sync.dma_start(b2p_t[:], b2pd[:])
                w2T_t = cpool.tile([A, C], bf16, name="w2T", tag="w2T")
                nc.sync.dma_start(w2T_t[:], w2T[:])
                ident_t = cpool.tile([128, 128], f32, name="ident", tag="ident")
                nc.sync.dma_start(ident_t[:], identd[:])
                wt = cpool.tile([128, 2 * NCH * A], f32, name="wmsall", tag="wmsall")
                src_ap = wmsT.rearrange("(k p) a -> p k a", p=128)
                nc.sync.dma_start(wt[:].rearrange("p (k a) -> p k a", a=A), src_ap)
                wms_t = [wt[:, k * A:(k + 1) * A] for k in range(2 * NCH)]

            def body():
                global w1xT_t
                # prologue: phase A of samples 0/1, weights interleaved,
                # sample 2's DMA prefetched
                dma_x(0, groups=[0])
                w1xT_t = load_w1xT()
                dma_x(0, groups=[1, 2, 3])
                for c in range(NCH):
                    phaseA_moments(0, c, sxx_dve=(c % 2 == 1))
                    phaseA_mm1(0, c)
                load_params()
                dma_x(1)
                for c in range(NCH):
                    phaseA_moments(1, c, sxx_dve=(c % 3 == 2))
                dma_x(2)
                phaseB(0)
                for c in range(NCH):
                    phaseA_mm1(1, c)
                # steady state: C(s) carries A(s+2) moments, B(s+1) at c6,
                # and A(s+2)'s matmul1 interleaved in the c>=7 shadow of
                # relu(s+1) freeing the ph1 slot.
                for s in range(SPC):
                    for c in range(NCH):
                        phaseC_chunk(s, c, s2_act=False)
                        if s + 3 < SPC and c == 0:
                            dma_x(s + 3)
                        if s + 2 < SPC and c < 8:
                            phaseA_moments(s + 2, c)
                        if s + 2 < SPC and c >= 8:
                            phaseA_moments(s + 2, c)
                            for cc in range(3 * (c - 8), 3 * (c - 8) + 3):
                                phaseA_mm1(s + 2, cc)
                        if c == 3 and s + 1 < SPC:
                            phaseB_stats(s + 1)
                        if c == 5 and s + 1 < SPC:
                            phaseB_main(s + 1)
                    sample_out(s)
                    del st[s]

            if loop_reps == 1:
                body()
            else:
                with tc.For_i(0, loop_reps, 1):
                    body()

    nc.compile()
    return nc


def _get_module(loop_reps=1):
    key = loop_reps
    if key not in _CACHE:
        _CACHE[key] = _build_module(loop_reps)
    return _CACHE[key]


def _host_prep(inputs):
    """Precompute folded parameters and shard inputs. Returns per-core in_maps."""
    x = np.asarray(inputs["x"])
    W1 = np.asarray(inputs["W1"], np.float32)
    b1 = np.asarray(inputs["b1"], np.float32)
    g1 = np.asarray(inputs["g1"], np.float32)
    beta1 = np.asarray(inputs["beta1"], np.float32)
    rm1 = np.asarray(inputs["rm1"], np.float32)
    rv1 = np.asarray(inputs["rv1"], np.float32)
    W2 = np.asarray(inputs["W2"], np.float32)
    b2 = np.asarray(inputs["b2"], np.float32)
    g2 = np.asarray(inputs["g2"], np.float32)
    rv2 = np.asarray(inputs["rv2"], np.float32)

    inv1 = (g1 / np.sqrt(rv1 + BN_EPS)).astype(np.float32)
    add1 = (beta1 - rm1 * inv1).astype(np.float32)
    inv2 = (g2 / np.sqrt(rv2 + BN_EPS)).astype(np.float32)
    b2p = (inv2 * b2).astype(np.float32)

    const = {
        "w1xT": np.ascontiguousarray(W1[:, :C].T).astype(ml_dtypes.bfloat16),
        "wmsT": np.ascontiguousarray(W1[:, C:].T).astype(np.float32),
        "w2T": np.ascontiguousarray(W2.T).astype(ml_dtypes.bfloat16),
        "b1d": b1.reshape(A, 1),
        "inv1d": inv1.reshape(A, 1),
        "add1d": add1.reshape(A, 1),
        "inv2d": np.ascontiguousarray(inv2.reshape(NCH, 128).T),
        "b2pd": np.ascontiguousarray(b2p.reshape(NCH, 128).T),
        "identd": np.eye(128, dtype=np.float32),
    }
    xbf = x.astype(ml_dtypes.bfloat16)
    in_maps = []
    for core in range(N_CORES):
        m = dict(const)
        m["xbf"] = np.ascontiguousarray(xbf[core * SPC:(core + 1) * SPC])
        in_maps.append(m)
    return in_maps


def kernel(**inputs):
    from concourse.bass_utils import run_bass_kernel_spmd

    nc = _get_module(loop_reps=1)
    in_maps = _host_prep(inputs)
    res = run_bass_kernel_spmd(nc, in_maps, core_ids=list(range(N_CORES)))
    out = np.concatenate([res.results[i]["out"] for i in range(N_CORES)], axis=0)
    return out.astype(np.float32)
